# revision 30
# baseline (speedup 1.0000x reference)
"""Trainium2 Bass kernel for weighted-CE + structural-penalty loss (v2).

Full inputs -> data-parallel shard over batch across 8 NeuronCores ->
per-core Bass kernel computes small partial sums -> host combines in
float64.

Decomposition (per core, positions laid out [128 partitions, 2048]):
  ce_sum = sum_pos wt*lse - sum_c w_c * Sc
    wt   = w[t] (host-gathered fp16, 8-entry table lookup)
    lse  = ln(sum_c exp(x_c)): ACT Exp (fp16, class-planar layout from a
           host-side transpose) -> DVE pairwise adds -> ACT Ln
    A    = sum wt*lse: DVE mult + PE ones-colsum into PSUM [1,512]
    Sc   = sum_{t=c} x_c: 8 per-class is_equal masks (DVE tensor_scalar
           4x vs per-partition fp32 scalars), Z = m*x (one packed
           tensor_tensor), PE ones-colsums into per-class PSUM rows.
  penalty (per row, codes Q = s + 4*s[+1] + 16*s[+2] + 64*s[+3] packed
  host-side as fp16):
    bracket part: d = (Q%4==1) - (Q%4==2); pf = sum d (ts accum);
      H = relu-scan of d (tensor_tensor_scan add/max vs zeros); host
      chains row halves via U(seed) = relu(Hb - pfb - Ha).
    pair part: indicators (Q%16==9 | Q%64==45 | Q%256==189) as single
      two-op tensor_scalars, PE colsums with lhsT columns 2/3/4 into one
      PSUM [1,512]; host adds the tiny clamped-tail correction.
"""

import numpy as np

import concourse.bass as bass
import concourse.mybir as mybir
import concourse.tile as tile
from concourse import bacc
from concourse.bass_utils import run_bass_kernel_spmd

B, S, C = 512, 4096, 8
PENALTY_WEIGHT = 0.1
NCORES = 8
RB = B // NCORES          # rows (batch) per core
P = 128                   # SBUF partitions
NP = (RB * S) // P        # positions per partition (2048)
NCH = 4                   # CE chunks
PCH = NP // NCH           # positions per partition per chunk (512)

F32 = mybir.dt.float32
F16 = mybir.dt.float16
OP = mybir.AluOpType
AF = mybir.ActivationFunctionType


def _patch_act_tables():
    """Prefer the single table set containing Exp+Ln+Copy so the kernel
    pays one ACT_TABLE_LOAD instead of alternating per chunk."""
    import concourse.hw_specs as hw_specs
    if getattr(hw_specs, "_loss_kernel_tables_patched", False):
        return
    orig = hw_specs.get_activation_tables

    def patched(arch):
        t = orig(arch)
        pref = "natural_log_exp_and_others"
        if pref not in t:
            return t
        return {k: (v if k == pref else set()) for k, v in t.items()}

    hw_specs.get_activation_tables = patched
    bacc.get_activation_tables = patched
    hw_specs._loss_kernel_tables_patched = True


def build_program(compile=True):
    _patch_act_tables()
    nc = bacc.Bacc("TRN2", target_bir_lowering=False, debug=False)

    x_d = nc.dram_tensor("xpl", [P, NCH * C * PCH], F16, kind="ExternalInput").ap()
    wt_d = nc.dram_tensor("wt", [P, NP], F16, kind="ExternalInput").ap()
    wv_d = nc.dram_tensor("wvals", [P, C], F32, kind="ExternalInput").ap()
    # structure code streams: s, s+4*s1, s+4*s1+16*s2, s+4*s1+16*s2+64*s3
    # (single-op is_equal tests; mod/bitwise fail the DVE ISA check).
    # One tensor, one DMA: fewer DGE descriptors.
    q_d = nc.dram_tensor("qc", [P, 4 * NP], F16, kind="ExternalInput").ap()

    # cols: pf (= sum d), H
    acc_d = nc.dram_tensor("accs", [P, 2], F32, kind="ExternalOutput").ap()
    # rows 0..7: per-class Sc colsums; row 8: A colsum; row 9: pen colsum
    sums_d = nc.dram_tensor("sums", [10, 512], F32, kind="ExternalOutput").ap()

    with tile.TileContext(nc) as tc:
        with (
            tc.tile_pool(name="const", bufs=1) as const,
            tc.tile_pool(name="xin", bufs=4) as xin,
            tc.tile_pool(name="ebuf", bufs=2) as ebuf,
            tc.tile_pool(name="mbuf", bufs=2) as mbuf,
            tc.tile_pool(name="zbuf", bufs=2) as zbuf,
            tc.tile_pool(name="mid", bufs=2) as mid,
            tc.tile_pool(name="pen", bufs=1) as pen,
            tc.tile_pool(name="acc", bufs=1) as acc,
            tc.tile_pool(name="psum", bufs=1, space="PSUM") as psum,
        ):
            # 3 colsum rows per PSUM bank (matmul outs must start at
            # partition 0/32/64): banks 0-2 hold [c0..c2], [c3..c5],
            # [c6, c7, A]; bank 3 holds the penalty colsum.
            ps_b = [psum.tile([65, 512], F32, name=f"ps_b{g}") for g in range(3)]
            ps_p = psum.tile([1, 512], F32, name="ps_p")

            def colsum_out(idx):
                return ps_b[idx // 3][(idx % 3) * 32 : (idx % 3) * 32 + 1, :]

            ones1 = const.tile([P, 1], F16)
            nc.gpsimd.memset(ones1, 1.0)
            penw = const.tile([P, 3], F16)
            for j, v in enumerate((2.0, 3.0, 4.0)):
                nc.gpsimd.memset(penw[:, j : j + 1], v)
            zer = const.tile([P, NP], F16)
            nc.gpsimd.memset(zer, 0.0)

            # DMA order: wt/wv first (masks need them), then the x chunks
            # (the pipeline), penalty code streams last.
            wt_sb = const.tile([P, NP], F16)
            nc.sync.dma_start(out=wt_sb, in_=wt_d)
            wv_sb = const.tile([P, C], F32)
            nc.sync.dma_start(out=wv_sb, in_=wv_d)

            acc_sb = acc.tile([P, 2], F32)

            lse_full = const.tile([P, NP], F16)

            # full-width per-class masks, up front (one DVE op per class)
            m_full = const.tile([P, C, NP], F16)
            for c in range(C):
                nc.vector.tensor_scalar(
                    out=m_full[:, c, :], in0=wt_sb,
                    scalar1=wv_sb[:, c : c + 1], scalar2=None,
                    op0=OP.is_equal)

            # ---------------- CE chunks ----------------
            x_ts = []
            for k in range(NCH):
                fl = k * C * PCH
                x_t = xin.tile([P, C, PCH], F16, tag="x")
                nc.sync.dma_start(out=x_t, in_=x_d[:, fl : fl + C * PCH])
                x_ts.append(x_t)
            q_all = const.tile([P, 4, NP], F16)
            nc.sync.dma_start(out=q_all, in_=q_d)
            q_sb = [q_all[:, j, :] for j in range(4)]

            nmm = NP // 512

            def pen_stream0():
                # lp/rp/d on DVE; pf accum on ACT; the relu-scan on gpsimd
                lp_t = pen.tile([P, NP], F16)
                nc.vector.tensor_scalar(out=lp_t, in0=q_sb[0], scalar1=1.0,
                                        scalar2=None, op0=OP.is_equal)
                rp_t = pen.tile([P, NP], F16)
                nc.vector.tensor_scalar(out=rp_t, in0=q_sb[0], scalar1=2.0,
                                        scalar2=None, op0=OP.is_equal)
                d_t = pen.tile([P, NP], F16)
                nc.vector.tensor_sub(d_t, lp_t, rp_t)
                djunk = pen.tile([P, NP], F16)
                nc.scalar.activation(djunk, d_t, AF.Copy,
                                     accum_out=acc_sb[:, 0:1])
                h_t = pen.tile([P, NP], F16)
                nc.vector.tensor_tensor_scan(out=h_t, data0=d_t, data1=zer,
                                             initial=0.0, op0=OP.add,
                                             op1=OP.max)
                nc.vector.tensor_copy(out=acc_sb[:, 1:2],
                                      in_=h_t[:, NP - 1 : NP])

            def pen_pairs(i, tv):
                pr = pen.tile([P, NP], F16, name=f"pr{i}")
                nc.vector.tensor_scalar(out=pr, in0=q_sb[i + 1], scalar1=tv,
                                        scalar2=None, op0=OP.is_equal)
                for w in range(nmm):
                    nc.tensor.matmul(ps_p, lhsT=penw[:, i : i + 1],
                                     rhs=pr[:, w * 512 : (w + 1) * 512],
                                     start=(i == 0 and w == 0),
                                     stop=(i == 2 and w == nmm - 1))

            pen_work = [pen_stream0,
                        lambda: pen_pairs(0, 9.0),
                        lambda: pen_pairs(1, 45.0),
                        lambda: pen_pairs(2, 189.0)]

            for k in range(NCH):
                x_t = x_ts[k]
                wtk = wt_sb[:, k * PCH : (k + 1) * PCH]

                e_t = ebuf.tile([P, C, PCH], F16, tag="e")
                nc.scalar.activation(e_t, x_t, AF.Exp)

                z_t = zbuf.tile([P, C, PCH], F16, tag="z")
                nc.vector.tensor_mul(
                    z_t, m_full[:, :, k * PCH : (k + 1) * PCH], x_t)

                t4 = mid.tile([P, 4, PCH], F16, tag="t4")
                nc.vector.tensor_add(t4, e_t[:, 0:4, :], e_t[:, 4:8, :])
                t2 = mid.tile([P, 2, PCH], F16, tag="t2")
                nc.vector.tensor_add(t2, t4[:, 0:2, :], t4[:, 2:4, :])
                se = mid.tile([P, PCH], F16, tag="se")
                nc.vector.tensor_add(se, t2[:, 0, :], t2[:, 1, :])
                nc.scalar.activation(lse_full[:, k * PCH : (k + 1) * PCH],
                                     se, AF.Ln)

                last = k == NCH - 1
                for c in range(C):
                    nc.tensor.matmul(colsum_out(c), lhsT=ones1,
                                     rhs=z_t[:, c, :],
                                     start=(k == 0), stop=last)
                pen_work[k]()

            # A-side: one full-width product, then 4 colsum matmuls
            prod = const.tile([P, NP], F16)
            nc.vector.tensor_mul(prod, wt_sb, lse_full)
            for w in range(NP // 512):
                nc.tensor.matmul(colsum_out(8), lhsT=ones1,
                                 rhs=prod[:, w * 512 : (w + 1) * 512],
                                 start=(w == 0), stop=(w == NP // 512 - 1))

            # ---------------- dumps ----------------
            nc.sync.dma_start(out=acc_d, in_=acc_sb)
            zs = [acc.tile([65, 512], F32, name=f"zs{g}") for g in range(3)]
            for g in range(3):
                nc.scalar.activation(zs[g], ps_b[g], AF.Copy)
                rows = bass.AP(tensor=zs[g].tensor, offset=zs[g].offset,
                               ap=[[zs[g].ap[0][0] * 32, 3], [1, 512]])
                nc.sync.dma_start(out=sums_d[g * 3 : g * 3 + 3, :], in_=rows)
            psb = acc.tile([1, 512], F32)
            nc.scalar.activation(psb, ps_p, AF.Copy)
            nc.sync.dma_start(out=sums_d[9:10, :], in_=psb)

    if compile:
        nc.compile()
    return nc


_program = None


def _get_program():
    global _program
    if _program is None:
        _program = build_program()
    return _program


def _unique_fp16(w):
    """fp16 weights, nudged to pairwise-distinct bit patterns."""
    wq = w.astype(np.float16)
    seen = set()
    for i in range(wq.shape[0]):
        v = wq[i]
        while v.tobytes() in seen:
            v = np.nextafter(v, np.float16(2.0), dtype=np.float16)
        seen.add(v.tobytes())
        wq[i] = v
    return wq


def make_in_maps(logits, targets, predicted_structures, ce_weights):
    t = np.asarray(targets, dtype=np.int64)
    s = np.asarray(predicted_structures).reshape(B, S).astype(np.int64)
    lg = np.asarray(logits, dtype=np.float32)
    wq = _unique_fp16(np.asarray(ce_weights, dtype=np.float64))
    wv32 = np.ascontiguousarray(
        np.broadcast_to(wq.astype(np.float32), (P, C)))

    i = np.arange(S)
    s1 = s[:, np.minimum(i + 1, S - 1)]
    s2 = s[:, np.minimum(i + 2, S - 1)]
    s3 = s[:, np.minimum(i + 3, S - 1)]
    qs = [s, s + 4 * s1, s + 4 * s1 + 16 * s2,
          s + 4 * s1 + 16 * s2 + 64 * s3]

    def split(a, dt):
        return np.ascontiguousarray(
            a.reshape(RB, 2, NP).transpose(1, 0, 2).reshape(P, NP)).astype(dt)

    in_maps = []
    for core in range(NCORES):
        rows = slice(core * RB, (core + 1) * RB)
        # row r -> partition r (pos 0..2047) and 64+r (pos 2048..4095)
        x_pp = lg[rows].reshape(RB, 2, NP, C).transpose(1, 0, 2, 3)
        x16 = np.ascontiguousarray(
            x_pp.reshape(P, NCH, PCH, C).transpose(0, 1, 3, 2)
        ).astype(np.float16).reshape(P, NCH * C * PCH)
        wt16 = np.ascontiguousarray(wq[split(t[rows], np.int64)])
        qc = np.stack([split(q[rows], np.float16) for q in qs],
                      axis=1).reshape(P, 4 * NP)
        in_maps.append({"xpl": x16, "wt": wt16, "wvals": wv32,
                        "qc": np.ascontiguousarray(qc)})
    return in_maps, t, s, wq


def combine_partials(results, t, s, ce_weights):
    w = np.asarray(ce_weights, np.float64)
    A = 0.0
    Sc = np.zeros(C, np.float64)
    pen = 0.0
    for r in results:
        sums = r["sums"].astype(np.float64)
        A += sums[8].sum()
        Sc += sums[0:8].sum(axis=1)
        pen += sums[9].sum()
        accs = r["accs"].astype(np.float64)
        pfa, ha = accs[0:RB, 0], accs[0:RB, 1]
        pfb, hb = accs[RB:P, 0], accs[RB:P, 1]
        ua = ha - pfa
        ub = np.maximum(hb - pfb - ha, 0.0)
        pen += ((pfa + pfb) + 2.0 * (ua + ub)).sum()

    # clamped-tail correction for pair3/pair4 (reference clamps dot offsets
    # at S-2; the device codes clamp uniformly at S-1)
    i = np.arange(S - 4, S)
    d1r = s[:, np.minimum(i + 1, S - 2)]
    d2r = s[:, np.minimum(i + 2, S - 2)]
    r1 = s[:, np.minimum(i + 1, S - 1)]
    r2 = s[:, np.minimum(i + 2, S - 1)]
    r3 = s[:, np.minimum(i + 3, S - 1)]
    lp = s[:, i] == 1
    ref_p3 = lp & (d1r == 3) & (r2 == 2)
    ref_p4 = lp & (d1r == 3) & (d2r == 3) & (r3 == 2)
    dev_p3 = lp & (r1 == 3) & (r2 == 2)
    dev_p4 = lp & (r1 == 3) & (r2 == 3) & (r3 == 2)
    pen += (3.0 * (ref_p3.astype(np.float64) - dev_p3)
            + 4.0 * (ref_p4.astype(np.float64) - dev_p4)).sum()

    nnz = float((t != 0).sum())
    ce = (A - (w * Sc).sum()) / (B * S)
    penalty = pen / nnz
    return np.float32(ce + PENALTY_WEIGHT * penalty)


def kernel(logits, targets, predicted_structures, ce_weights):
    in_maps, t, s, wq = make_in_maps(
        logits, targets, predicted_structures, ce_weights)
    nc = _get_program()
    res = run_bass_kernel_spmd(nc, in_maps, core_ids=list(range(NCORES)))
    return combine_partials(res.results, t, s, ce_weights)
